# revision 15
# baseline (speedup 1.0000x reference)
"""Trainium2 Bass kernel for nn_ALTER2Layer (4-layer tied-weight sigmoid
autoencoder forward + per-sample Jacobian), data-parallel over batch on
8 NeuronCores.

Math (per sample b), reassociated from the reference to minimize FLOPs:
    c1 = sig(x W1^T + b1); c2 = sig(c1 W2^T + b2); c3 = sig(c2 W2 + b3)
    recover = sig(c3 W1 + b_r)
    sK = cK (1 - cK)
    Jst[b] = W1^T diag(s1) W2^T diag(s2) W2 diag(s3) W1
           = LT[b]^T @ R[b]
      LT[b] = (diag(s2) W2 diag(s1)) @ W1        # [H2, D]
      R[b]  = (W2 diag(s3)) @ W1                 # [H2, D]
    Jac = permute(Jst) per the reference's cat/reshape.

Each core handles B/8 = 16 samples; weights are replicated. No collectives.
Weights ship pre-transposed and pre-cast (f16 forward, bf16 Jacobian) so the
device does no staging work; activations/sigmoid/diag scales stay f32.
"""

import os
import sys

sys.path.insert(0, "/opt/trn_rl_repo")

import numpy as np

B, D, H1, H2 = 128, 1024, 512, 256
NCORES = 8
BS = B // NCORES  # samples per core = 16

# Jacobian matmul dtype: "bf16" (fastest measured), "f16".
MM_MODE = os.environ.get("KERNEL_MM_MODE", "bf16")
# Forward-chain matmul dtype: "f16" (accurate, same speed at N=16), "bf16".
FWD_MODE = os.environ.get("KERNEL_FWD_MODE", "f16")
# Jacobian DRAM output dtype: "f32", "f16", "bf16".
JAC_OUT = os.environ.get("KERNEL_JAC_OUT", "bf16")

_cache = {}


def _np_dt(mode):
    import ml_dtypes
    return {"bf16": ml_dtypes.bfloat16, "f16": np.float16,
            "f32": np.float32}[mode]


def _build():
    from concourse import bacc, mybir, tile

    f32 = mybir.dt.float32
    SIG = mybir.ActivationFunctionType.Sigmoid
    COPY = mybir.ActivationFunctionType.Copy

    dt_map = {"bf16": mybir.dt.bfloat16, "f16": mybir.dt.float16, "f32": f32}
    mm_dt = dt_map[MM_MODE]
    fwd_dt = dt_map[FWD_MODE]
    jac_dt = dt_map[JAC_OUT]

    KH = H1 // 128  # 4: k-chunks over H1
    KD = D // 128   # 8: chunks over D
    ME = H2 // 128  # 2: chunks over H2
    NN = D // 512   # 2: 512-wide n-halves of D

    nc = bacc.Bacc("TRN2", target_bir_lowering=False, debug=False,
                   num_devices=NCORES)

    # ---- DRAM parameters (host-prepped layouts/dtypes, per-core x shard) --
    xt_d = nc.dram_tensor("xt", [D, BS], fwd_dt, kind="ExternalInput").ap()
    w1t_d = nc.dram_tensor("w1t", [D, H1], fwd_dt, kind="ExternalInput").ap()
    w2t_d = nc.dram_tensor("w2t", [H1, H2], fwd_dt, kind="ExternalInput").ap()
    w2_d = nc.dram_tensor("w2", [H2, H1], fwd_dt, kind="ExternalInput").ap()
    # biases packed [b1|b2|b3|b_r] -> [2304, 1]
    bz_d = nc.dram_tensor("bz", [H1 + H2 + H1 + D, 1], f32,
                          kind="ExternalInput").ap()

    f16 = mybir.dt.float16
    rec_d = nc.dram_tensor("rec", [D, BS], f16, kind="ExternalOutput").ap()
    c2o_d = nc.dram_tensor("c2o", [H2, BS], f16, kind="ExternalOutput").ap()
    jac_d = nc.dram_tensor("jac", [BS, D, D], jac_dt, kind="ExternalOutput").ap()

    def part(ap, p=128):
        # [(a p), rest...] DRAM view -> [p, a, rest...]
        return ap.rearrange("(a p) d -> p a d", p=p)

    with tile.TileContext(nc) as tc:
        with (
            tc.tile_pool(name="const", bufs=1) as const,
            tc.tile_pool(name="fwd", bufs=1) as fwd,
            tc.tile_pool(name="ab", bufs=2) as ab_pool,
            tc.tile_pool(name="lr", bufs=2) as lr_pool,
            tc.tile_pool(name="jo", bufs=4) as jo_pool,
        ):
            # ---------- constants: chunked DMAs ordered by first use -------
            from concourse.masks import make_identity
            xt = const.tile([128, KD, BS], fwd_dt)       # x^T
            nc.sync.dma_start(out=xt[:], in_=part(xt_d))
            bz = const.tile([128, 18, 1], f32)           # packed biases
            nc.sync.dma_start(out=bz[:], in_=part(bz_d))
            b1s, b2s = bz[:, 0:KH], bz[:, KH:KH + 2]
            b3s, brs = bz[:, KH + 2:KH + 6], bz[:, KH + 6:KH + 14]
            w1t = const.tile([128, KD, H1], fwd_dt)      # W1^T
            nc.sync.dma_start(out=w1t[:], in_=part(w1t_d))
            w2t = const.tile([128, KH, H2], fwd_dt)      # W2^T
            nc.sync.dma_start(out=w2t[:], in_=part(w2t_d))
            w2 = const.tile([128, ME, H1], fwd_dt)       # W2
            nc.sync.dma_start(out=w2[:], in_=part(w2_d))
            ident = const.tile([128, 128], fwd_dt)
            make_identity(nc, ident[:])
            # W1 in Jacobian dtype, derived on-device: w1r = (W1^T)^T
            w1r = const.tile([128, KH, D], mm_dt)

            # ---------------- forward chain ----------------
            # sigmoid writes 16-bit activation tiles directly; f32 s-tiles
            # (diag scales) derive from them off the layer critical path
            s1t = fwd.tile([128, KH, BS], f32)
            s2t = fwd.tile([128, H2 // 128, BS], f32)
            s3t = fwd.tile([128, KH, BS], f32)
            rect = fwd.tile([128, KD, BS], f16)
            tmp = fwd.tile([128, KD, BS], f32)
            c1tb = fwd.tile([128, KH, BS], fwd_dt)
            c2tb = fwd.tile([128, H2 // 128, BS], fwd_dt)
            c3tb = fwd.tile([128, KH, BS], mm_dt)  # recover pairs with w1r
            warm = fwd.tile([128, 1], f32)
            nc.vector.memset(warm[:], 0.0)
            nc.scalar.activation(warm[:], warm[:], SIG)  # load sigmoid table

            # c1^T[h,b] = sig(sum_d W1T[d,h] xt[d,b] + b1[h])
            # k-outer so accumulation + the w1r transposes overlap the
            # streaming w1t DMA chunk by chunk
            with (
                tc.tile_pool(name="pc1", bufs=1, space="PSUM") as pc1,
                tc.tile_pool(name="ptr", bufs=3, space="PSUM") as ptr,
            ):
                pfs = [pc1.tile([128, BS], f32, name=f"pf1_{m}")
                       for m in range(KH)]
                for k in range(KD):
                    for m in range(KH):
                        nc.tensor.matmul(
                            pfs[m], w1t[:, k, m * 128:(m + 1) * 128],
                            xt[:, k, :],
                            start=(k == 0), stop=(k == KD - 1))
                    # w1r[h, d] = w1t[d, h]^T via PE transposes
                    for hk in range(KH):
                        pt = ptr.tile([128, 128], fwd_dt)
                        nc.tensor.transpose(
                            pt[:], w1t[:, k, hk * 128:(hk + 1) * 128],
                            ident[:])
                        nc.vector.tensor_copy(
                            w1r[:, hk, k * 128:(k + 1) * 128], pt[:])
                for m in range(KH):
                    nc.scalar.activation(c1tb[:, m, :], pfs[m], SIG,
                                         bias=b1s[:, m, :])
                # s1 = c1 - c1^2
                nc.vector.tensor_mul(tmp[:, :KH, :], c1tb[:], c1tb[:])
                nc.vector.tensor_sub(s1t[:], c1tb[:], tmp[:, :KH, :])

            with tc.tile_pool(name="pfwd", bufs=2, space="PSUM") as pfwd:

                # c2^T[e,b] = sig(sum_h W2T[h,e] c1t[h,b] + b2[e])
                for m in range(H2 // 128):
                    pf = pfwd.tile([128, BS], f32)
                    for k in range(KH):
                        nc.tensor.matmul(
                            pf, w2t[:, k, m * 128:(m + 1) * 128], c1tb[:, k, :],
                            start=(k == 0), stop=(k == KH - 1))
                    nc.scalar.activation(c2tb[:, m, :], pf, SIG,
                                         bias=b2s[:, m, :])
                nc.vector.tensor_mul(tmp[:, :H2 // 128, :], c2tb[:], c2tb[:])
                nc.vector.tensor_sub(s2t[:], c2tb[:], tmp[:, :H2 // 128, :])
                nc.sync.dma_start(out=part(c2o_d), in_=c2tb[:])

                # c3^T[h,b] = sig(sum_e W2[e,h] c2t[e,b] + b3[h])
                for m in range(KH):
                    pf = pfwd.tile([128, BS], f32)
                    for k in range(ME):
                        nc.tensor.matmul(
                            pf, w2[:, k, m * 128:(m + 1) * 128], c2tb[:, k, :],
                            start=(k == 0), stop=(k == ME - 1))
                    nc.scalar.activation(c3tb[:, m, :], pf, SIG,
                                         bias=b3s[:, m, :])
                nc.vector.tensor_mul(tmp[:, :KH, :], c3tb[:], c3tb[:])
                nc.vector.tensor_sub(s3t[:], c3tb[:], tmp[:, :KH, :])

                # recover^T[d,b] = sig(sum_h W1[h,d] c3t[h,b] + br[d])
                for m in range(KD):
                    pf = pfwd.tile([128, BS], f32)
                    for k in range(KH):
                        nc.tensor.matmul(
                            pf, w1r[:, k, m * 128:(m + 1) * 128],
                            c3tb[:, k, :],
                            start=(k == 0), stop=(k == KH - 1))
                    nc.scalar.activation(rect[:, m, :], pf, SIG,
                                         bias=brs[:, m, :])
                nc.sync.dma_start(out=part(rec_d), in_=rect[:])

            # ---------------- Jacobian, software-pipelined over samples ----
            with (
                tc.tile_pool(name="pprep", bufs=3, space="PSUM") as pprep,
                tc.tile_pool(name="pout", bufs=5, space="PSUM") as pout,
            ):
                def prep(s):
                    """Build LT[s] and R[s]: [H2, D] each, SBUF, mm dtype."""
                    at = ab_pool.tile([128, KH, H2], mm_dt, tag="at")
                    bt = ab_pool.tile([128, KH, H2], mm_dt, tag="bt")
                    for k in range(KH):
                        nc.vector.tensor_scalar_mul(
                            at[:, k, :], w2t[:, k, :], s1t[:, k, s:s + 1])
                        nc.vector.tensor_scalar_mul(
                            bt[:, k, :], w2t[:, k, :], s3t[:, k, s:s + 1])
                    lt = lr_pool.tile([128, ME, D], mm_dt, tag="lt")
                    rt = lr_pool.tile([128, ME, D], mm_dt, tag="rt")
                    for m in range(ME):
                        for n in range(NN):
                            pp = pprep.tile([128, 512], f32)
                            for k in range(KH):
                                nc.tensor.matmul(
                                    pp,
                                    at[:, k, m * 128:(m + 1) * 128],
                                    w1r[:, k, n * 512:(n + 1) * 512],
                                    start=(k == 0), stop=(k == KH - 1))
                            # row-scale by s2 fused into the PSUM->SBUF copy
                            nc.scalar.activation(
                                lt[:, m, n * 512:(n + 1) * 512], pp, COPY,
                                scale=s2t[:, m, s:s + 1])
                        for n in range(NN):
                            pp = pprep.tile([128, 512], f32)
                            for k in range(KH):
                                nc.tensor.matmul(
                                    pp,
                                    bt[:, k, m * 128:(m + 1) * 128],
                                    w1r[:, k, n * 512:(n + 1) * 512],
                                    start=(k == 0), stop=(k == KH - 1))
                            nc.vector.tensor_copy(
                                rt[:, m, n * 512:(n + 1) * 512], pp)
                    return lt, rt

                def final(s, lt, rt):
                    """Jst[s] = LT[s]^T @ R[s] -> DRAM jac[s]."""
                    for m in range(KD):
                        jout = jo_pool.tile([128, D], jac_dt, tag="jout")
                        for n in range(NN):
                            po = pout.tile([128, 512], f32)
                            for k in range(ME):
                                nc.tensor.matmul(
                                    po,
                                    lt[:, k, m * 128:(m + 1) * 128],
                                    rt[:, k, n * 512:(n + 1) * 512],
                                    start=(k == 0), stop=(k == ME - 1))
                            # alternate copy engine to balance ACT/DVE load
                            dst = jout[:, n * 512:(n + 1) * 512]
                            if (m + n) % 2 == 0:
                                nc.scalar.activation(dst, po, COPY)
                            else:
                                nc.vector.tensor_copy(dst, po)
                        nc.sync.dma_start(
                            out=jac_d[s, m * 128:(m + 1) * 128, :], in_=jout[:])

                lt, rt = prep(0)
                for s in range(BS):
                    nxt = prep(s + 1) if s + 1 < BS else None
                    final(s, lt, rt)
                    if nxt is not None:
                        lt, rt = nxt

    nc.compile()
    return nc


def _get_nc():
    key = (MM_MODE, FWD_MODE, JAC_OUT)
    if key not in _cache:
        _cache[key] = _build()
    return _cache[key]


def _ensure_profile_hook():
    """Install antenv.axon_hooks + the NTFF ctypes hook if the image lacks
    them (needed only for trace=True; degrades silently)."""
    try:
        from antenv.axon_hooks import get_axon_ntff_profile_hook  # noqa: F401
        return
    except ImportError:
        pass
    import contextlib
    import ctypes
    import types

    try:
        import antenv
    except ImportError:
        return
    mod = types.ModuleType("antenv.axon_hooks")
    mod._hook = None
    mod.set_axon_ntff_profile_hook = lambda h: setattr(mod, "_hook", h)
    mod.get_axon_ntff_profile_hook = lambda: mod._hook
    sys.modules["antenv.axon_hooks"] = mod
    antenv.axon_hooks = mod
    try:
        lib = ctypes.CDLL("/opt/axon/libaxon_pjrt.so")
        if not hasattr(lib, "axon_start_nrt_profile"):
            return
        lib.axon_start_nrt_profile.argtypes = [
            ctypes.POINTER(ctypes.c_int64), ctypes.c_size_t]
        lib.axon_start_nrt_profile.restype = ctypes.c_int64
        lib.axon_stop_nrt_profile.argtypes = [ctypes.c_char_p]
        lib.axon_stop_nrt_profile.restype = ctypes.c_int64

        @contextlib.contextmanager
        def _hook(output_dir, device_ids):
            import jax
            jax.devices()
            if device_ids:
                ids = (ctypes.c_int64 * len(device_ids))(*device_ids)
                rc = lib.axon_start_nrt_profile(ids, len(device_ids))
            else:
                rc = lib.axon_start_nrt_profile(None, 0)
            if rc != 0:
                raise RuntimeError(f"axon_start_nrt_profile rc={rc}")
            try:
                yield
            finally:
                n = lib.axon_stop_nrt_profile(str(output_dir).encode())
                print(f"profile: {n} file(s) written to {output_dir}",
                      file=sys.stderr)

        mod._hook = _hook
    except Exception:
        mod._hook = None


def kernel(x, W1, b1, W2, b2, b3, b_r, _want_results=False):
    from concourse.bass_utils import run_bass_kernel_spmd

    nc = _get_nc()
    fwd_np = _np_dt(FWD_MODE)

    x = np.asarray(x, np.float32)
    W1 = np.asarray(W1, np.float32)
    W2 = np.asarray(W2, np.float32)
    w1t = np.ascontiguousarray(W1.T).astype(fwd_np)
    w2t = np.ascontiguousarray(W2.T).astype(fwd_np)
    w2f = W2.astype(fwd_np)
    bz = np.concatenate([
        np.asarray(b1, np.float32).ravel(), np.asarray(b2, np.float32).ravel(),
        np.asarray(b3, np.float32).ravel(), np.asarray(b_r, np.float32).ravel(),
    ]).reshape(-1, 1)

    in_maps = []
    for i in range(NCORES):
        xs = np.ascontiguousarray(x[i * BS:(i + 1) * BS].T).astype(fwd_np)
        in_maps.append({
            "xt": xs, "w1t": w1t, "w2t": w2t, "w2": w2f, "bz": bz,
        })

    trace = bool(os.environ.get("KERNEL_TRACE"))
    if trace:
        _ensure_profile_hook()
    res = run_bass_kernel_spmd(nc, in_maps, core_ids=list(range(NCORES)),
                               trace=trace)

    rec = np.empty((B, D), np.float32)
    c2 = np.empty((B, H2), np.float32)
    Jst = np.empty((B, D, D), np.float32)
    for i in range(NCORES):
        r = res.results[i]
        rec[i * BS:(i + 1) * BS] = r["rec"].T.astype(np.float32)
        c2[i * BS:(i + 1) * BS] = r["c2o"].T.astype(np.float32)
        Jst[i * BS:(i + 1) * BS] = r["jac"].astype(np.float32)

    # reference's cat/reshape permutation
    Jac = np.transpose(Jst, (1, 0, 2)).reshape(D, B * D).reshape(B, D, D)
    out = (rec, c2, Jac)
    if _want_results:
        return out, res
    return out


# revision 16
# speedup vs baseline: 1.0166x; 1.0166x over previous
"""Trainium2 Bass kernel for nn_ALTER2Layer (4-layer tied-weight sigmoid
autoencoder forward + per-sample Jacobian), data-parallel over batch on
8 NeuronCores.

Math (per sample b), reassociated from the reference to minimize FLOPs:
    c1 = sig(x W1^T + b1); c2 = sig(c1 W2^T + b2); c3 = sig(c2 W2 + b3)
    recover = sig(c3 W1 + b_r)
    sK = cK (1 - cK)
    Jst[b] = W1^T diag(s1) W2^T diag(s2) W2 diag(s3) W1
           = LT[b]^T @ R[b]
      LT[b] = (diag(s2) W2 diag(s1)) @ W1        # [H2, D]
      R[b]  = (W2 diag(s3)) @ W1                 # [H2, D]
    Jac = permute(Jst) per the reference's cat/reshape.

Each core handles B/8 = 16 samples; weights are replicated. No collectives.
Weights ship pre-transposed and pre-cast (f16 forward, bf16 Jacobian) so the
device does no staging work; activations/sigmoid/diag scales stay f32.
"""

import os
import sys

sys.path.insert(0, "/opt/trn_rl_repo")

import numpy as np

B, D, H1, H2 = 128, 1024, 512, 256
NCORES = 8
BS = B // NCORES  # samples per core = 16

# Jacobian matmul dtype: "bf16" (fastest measured), "f16".
MM_MODE = os.environ.get("KERNEL_MM_MODE", "bf16")
# Forward-chain matmul dtype: "f16" (accurate, same speed at N=16), "bf16".
FWD_MODE = os.environ.get("KERNEL_FWD_MODE", "f16")
# Jacobian DRAM output dtype: "f32", "f16", "bf16".
JAC_OUT = os.environ.get("KERNEL_JAC_OUT", "bf16")

_cache = {}


def _np_dt(mode):
    import ml_dtypes
    return {"bf16": ml_dtypes.bfloat16, "f16": np.float16,
            "f32": np.float32}[mode]


def _build():
    from concourse import bacc, mybir, tile

    f32 = mybir.dt.float32
    SIG = mybir.ActivationFunctionType.Sigmoid
    COPY = mybir.ActivationFunctionType.Copy

    dt_map = {"bf16": mybir.dt.bfloat16, "f16": mybir.dt.float16, "f32": f32}
    mm_dt = dt_map[MM_MODE]
    fwd_dt = dt_map[FWD_MODE]
    jac_dt = dt_map[JAC_OUT]

    KH = H1 // 128  # 4: k-chunks over H1
    KD = D // 128   # 8: chunks over D
    ME = H2 // 128  # 2: chunks over H2
    NN = D // 512   # 2: 512-wide n-halves of D

    nc = bacc.Bacc("TRN2", target_bir_lowering=False, debug=False,
                   num_devices=NCORES)

    # ---- DRAM parameters (host-prepped layouts/dtypes, per-core x shard) --
    xt_d = nc.dram_tensor("xt", [D, BS], fwd_dt, kind="ExternalInput").ap()
    w1t_d = nc.dram_tensor("w1t", [D, H1], fwd_dt, kind="ExternalInput").ap()
    w2t_d = nc.dram_tensor("w2t", [H1, H2], fwd_dt, kind="ExternalInput").ap()
    w2_d = nc.dram_tensor("w2", [H2, H1], fwd_dt, kind="ExternalInput").ap()
    # biases packed [b1|b2|b3|b_r] -> [2304, 1]
    bz_d = nc.dram_tensor("bz", [H1 + H2 + H1 + D, 1], f32,
                          kind="ExternalInput").ap()

    f16 = mybir.dt.float16
    rec_d = nc.dram_tensor("rec", [D, BS], f16, kind="ExternalOutput").ap()
    c2o_d = nc.dram_tensor("c2o", [H2, BS], f16, kind="ExternalOutput").ap()
    jac_d = nc.dram_tensor("jac", [BS, D, D], jac_dt, kind="ExternalOutput").ap()

    def part(ap, p=128):
        # [(a p), rest...] DRAM view -> [p, a, rest...]
        return ap.rearrange("(a p) d -> p a d", p=p)

    with tile.TileContext(nc) as tc:
        with (
            tc.tile_pool(name="const", bufs=1) as const,
            tc.tile_pool(name="fwd", bufs=1) as fwd,
            tc.tile_pool(name="ab", bufs=3) as ab_pool,
            tc.tile_pool(name="lr", bufs=3) as lr_pool,
            tc.tile_pool(name="jo", bufs=6) as jo_pool,
        ):
            # ---------- constants: chunked DMAs ordered by first use -------
            from concourse.masks import make_identity
            xt = const.tile([128, KD, BS], fwd_dt)       # x^T
            nc.sync.dma_start(out=xt[:], in_=part(xt_d))
            bz = const.tile([128, 18, 1], f32)           # packed biases
            nc.sync.dma_start(out=bz[:], in_=part(bz_d))
            b1s, b2s = bz[:, 0:KH], bz[:, KH:KH + 2]
            b3s, brs = bz[:, KH + 2:KH + 6], bz[:, KH + 6:KH + 14]
            w1t = const.tile([128, KD, H1], fwd_dt)      # W1^T
            nc.sync.dma_start(out=w1t[:], in_=part(w1t_d))
            w2t = const.tile([128, KH, H2], fwd_dt)      # W2^T
            nc.sync.dma_start(out=w2t[:], in_=part(w2t_d))
            w2 = const.tile([128, ME, H1], fwd_dt)       # W2
            nc.sync.dma_start(out=w2[:], in_=part(w2_d))
            ident = const.tile([128, 128], fwd_dt)
            make_identity(nc, ident[:])
            # W1 in Jacobian dtype, derived on-device: w1r = (W1^T)^T
            w1r = const.tile([128, KH, D], mm_dt)

            # ---------------- forward chain ----------------
            # sigmoid writes 16-bit activation tiles directly; f32 s-tiles
            # (diag scales) derive from them off the layer critical path
            s1t = fwd.tile([128, KH, BS], f32)
            s2t = fwd.tile([128, H2 // 128, BS], f32)
            s3t = fwd.tile([128, KH, BS], f32)
            rect = fwd.tile([128, KD, BS], f16)
            tmp = fwd.tile([128, KD, BS], f32)
            c1tb = fwd.tile([128, KH, BS], fwd_dt)
            c2tb = fwd.tile([128, H2 // 128, BS], fwd_dt)
            c3tb = fwd.tile([128, KH, BS], mm_dt)  # recover pairs with w1r
            warm = fwd.tile([128, 1], f32)
            nc.vector.memset(warm[:], 0.0)
            nc.scalar.activation(warm[:], warm[:], SIG)  # load sigmoid table

            # c1^T[h,b] = sig(sum_d W1T[d,h] xt[d,b] + b1[h])
            # k-outer so accumulation + the w1r transposes overlap the
            # streaming w1t DMA chunk by chunk
            with (
                tc.tile_pool(name="pc1", bufs=1, space="PSUM") as pc1,
                tc.tile_pool(name="ptr", bufs=3, space="PSUM") as ptr,
            ):
                pfs = [pc1.tile([128, BS], f32, name=f"pf1_{m}")
                       for m in range(KH)]
                for k in range(KD):
                    for m in range(KH):
                        nc.tensor.matmul(
                            pfs[m], w1t[:, k, m * 128:(m + 1) * 128],
                            xt[:, k, :],
                            start=(k == 0), stop=(k == KD - 1))
                    # w1r[h, d] = w1t[d, h]^T via PE transposes
                    for hk in range(KH):
                        pt = ptr.tile([128, 128], fwd_dt)
                        nc.tensor.transpose(
                            pt[:], w1t[:, k, hk * 128:(hk + 1) * 128],
                            ident[:])
                        nc.vector.tensor_copy(
                            w1r[:, hk, k * 128:(k + 1) * 128], pt[:])
                for m in range(KH):
                    nc.scalar.activation(c1tb[:, m, :], pfs[m], SIG,
                                         bias=b1s[:, m, :])
                # s1 = c1 - c1^2
                nc.vector.tensor_mul(tmp[:, :KH, :], c1tb[:], c1tb[:])
                nc.vector.tensor_sub(s1t[:], c1tb[:], tmp[:, :KH, :])

            with tc.tile_pool(name="pfwd", bufs=2, space="PSUM") as pfwd:

                # c2^T[e,b] = sig(sum_h W2T[h,e] c1t[h,b] + b2[e])
                for m in range(H2 // 128):
                    pf = pfwd.tile([128, BS], f32)
                    for k in range(KH):
                        nc.tensor.matmul(
                            pf, w2t[:, k, m * 128:(m + 1) * 128], c1tb[:, k, :],
                            start=(k == 0), stop=(k == KH - 1))
                    nc.scalar.activation(c2tb[:, m, :], pf, SIG,
                                         bias=b2s[:, m, :])
                nc.vector.tensor_mul(tmp[:, :H2 // 128, :], c2tb[:], c2tb[:])
                nc.vector.tensor_sub(s2t[:], c2tb[:], tmp[:, :H2 // 128, :])
                nc.sync.dma_start(out=part(c2o_d), in_=c2tb[:])

                # c3^T[h,b] = sig(sum_e W2[e,h] c2t[e,b] + b3[h])
                for m in range(KH):
                    pf = pfwd.tile([128, BS], f32)
                    for k in range(ME):
                        nc.tensor.matmul(
                            pf, w2[:, k, m * 128:(m + 1) * 128], c2tb[:, k, :],
                            start=(k == 0), stop=(k == ME - 1))
                    nc.scalar.activation(c3tb[:, m, :], pf, SIG,
                                         bias=b3s[:, m, :])
                nc.vector.tensor_mul(tmp[:, :KH, :], c3tb[:], c3tb[:])
                nc.vector.tensor_sub(s3t[:], c3tb[:], tmp[:, :KH, :])

                # recover^T[d,b] = sig(sum_h W1[h,d] c3t[h,b] + br[d])
                for m in range(KD):
                    pf = pfwd.tile([128, BS], f32)
                    for k in range(KH):
                        nc.tensor.matmul(
                            pf, w1r[:, k, m * 128:(m + 1) * 128],
                            c3tb[:, k, :],
                            start=(k == 0), stop=(k == KH - 1))
                    nc.scalar.activation(rect[:, m, :], pf, SIG,
                                         bias=brs[:, m, :])
                nc.sync.dma_start(out=part(rec_d), in_=rect[:])

            # ---------------- Jacobian, software-pipelined over samples ----
            with (
                tc.tile_pool(name="pprep", bufs=3, space="PSUM") as pprep,
                tc.tile_pool(name="pout", bufs=5, space="PSUM") as pout,
            ):
                def prep(s):
                    """Build LT[s] and R[s]: [H2, D] each, SBUF, mm dtype."""
                    at = ab_pool.tile([128, KH, H2], mm_dt, tag="at")
                    bt = ab_pool.tile([128, KH, H2], mm_dt, tag="bt")
                    for k in range(KH):
                        nc.vector.tensor_scalar_mul(
                            at[:, k, :], w2t[:, k, :], s1t[:, k, s:s + 1])
                        nc.vector.tensor_scalar_mul(
                            bt[:, k, :], w2t[:, k, :], s3t[:, k, s:s + 1])
                    lt = lr_pool.tile([128, ME, D], mm_dt, tag="lt")
                    rt = lr_pool.tile([128, ME, D], mm_dt, tag="rt")
                    for m in range(ME):
                        for n in range(NN):
                            pp = pprep.tile([128, 512], f32)
                            for k in range(KH):
                                nc.tensor.matmul(
                                    pp,
                                    at[:, k, m * 128:(m + 1) * 128],
                                    w1r[:, k, n * 512:(n + 1) * 512],
                                    start=(k == 0), stop=(k == KH - 1))
                            # row-scale by s2 fused into the PSUM->SBUF copy
                            nc.scalar.activation(
                                lt[:, m, n * 512:(n + 1) * 512], pp, COPY,
                                scale=s2t[:, m, s:s + 1])
                        for n in range(NN):
                            pp = pprep.tile([128, 512], f32)
                            for k in range(KH):
                                nc.tensor.matmul(
                                    pp,
                                    bt[:, k, m * 128:(m + 1) * 128],
                                    w1r[:, k, n * 512:(n + 1) * 512],
                                    start=(k == 0), stop=(k == KH - 1))
                            nc.vector.tensor_copy(
                                rt[:, m, n * 512:(n + 1) * 512], pp)
                    return lt, rt

                def final(s, lt, rt):
                    """Jst[s] = LT[s]^T @ R[s] -> DRAM jac[s]."""
                    for m in range(KD):
                        jout = jo_pool.tile([128, D], jac_dt, tag="jout")
                        for n in range(NN):
                            po = pout.tile([128, 512], f32)
                            for k in range(ME):
                                nc.tensor.matmul(
                                    po,
                                    lt[:, k, m * 128:(m + 1) * 128],
                                    rt[:, k, n * 512:(n + 1) * 512],
                                    start=(k == 0), stop=(k == ME - 1))
                            # alternate copy engine to balance ACT/DVE load
                            dst = jout[:, n * 512:(n + 1) * 512]
                            if (m + n) % 2 == 0:
                                nc.scalar.activation(dst, po, COPY)
                            else:
                                nc.vector.tensor_copy(dst, po)
                        nc.sync.dma_start(
                            out=jac_d[s, m * 128:(m + 1) * 128, :], in_=jout[:])

                lt, rt = prep(0)
                for s in range(BS):
                    nxt = prep(s + 1) if s + 1 < BS else None
                    final(s, lt, rt)
                    if nxt is not None:
                        lt, rt = nxt

    nc.compile()
    return nc


def _get_nc():
    key = (MM_MODE, FWD_MODE, JAC_OUT)
    if key not in _cache:
        _cache[key] = _build()
    return _cache[key]


def _ensure_profile_hook():
    """Install antenv.axon_hooks + the NTFF ctypes hook if the image lacks
    them (needed only for trace=True; degrades silently)."""
    try:
        from antenv.axon_hooks import get_axon_ntff_profile_hook  # noqa: F401
        return
    except ImportError:
        pass
    import contextlib
    import ctypes
    import types

    try:
        import antenv
    except ImportError:
        return
    mod = types.ModuleType("antenv.axon_hooks")
    mod._hook = None
    mod.set_axon_ntff_profile_hook = lambda h: setattr(mod, "_hook", h)
    mod.get_axon_ntff_profile_hook = lambda: mod._hook
    sys.modules["antenv.axon_hooks"] = mod
    antenv.axon_hooks = mod
    try:
        lib = ctypes.CDLL("/opt/axon/libaxon_pjrt.so")
        if not hasattr(lib, "axon_start_nrt_profile"):
            return
        lib.axon_start_nrt_profile.argtypes = [
            ctypes.POINTER(ctypes.c_int64), ctypes.c_size_t]
        lib.axon_start_nrt_profile.restype = ctypes.c_int64
        lib.axon_stop_nrt_profile.argtypes = [ctypes.c_char_p]
        lib.axon_stop_nrt_profile.restype = ctypes.c_int64

        @contextlib.contextmanager
        def _hook(output_dir, device_ids):
            import jax
            jax.devices()
            if device_ids:
                ids = (ctypes.c_int64 * len(device_ids))(*device_ids)
                rc = lib.axon_start_nrt_profile(ids, len(device_ids))
            else:
                rc = lib.axon_start_nrt_profile(None, 0)
            if rc != 0:
                raise RuntimeError(f"axon_start_nrt_profile rc={rc}")
            try:
                yield
            finally:
                n = lib.axon_stop_nrt_profile(str(output_dir).encode())
                print(f"profile: {n} file(s) written to {output_dir}",
                      file=sys.stderr)

        mod._hook = _hook
    except Exception:
        mod._hook = None


def kernel(x, W1, b1, W2, b2, b3, b_r, _want_results=False):
    from concourse.bass_utils import run_bass_kernel_spmd

    nc = _get_nc()
    fwd_np = _np_dt(FWD_MODE)

    x = np.asarray(x, np.float32)
    W1 = np.asarray(W1, np.float32)
    W2 = np.asarray(W2, np.float32)
    w1t = np.ascontiguousarray(W1.T).astype(fwd_np)
    w2t = np.ascontiguousarray(W2.T).astype(fwd_np)
    w2f = W2.astype(fwd_np)
    bz = np.concatenate([
        np.asarray(b1, np.float32).ravel(), np.asarray(b2, np.float32).ravel(),
        np.asarray(b3, np.float32).ravel(), np.asarray(b_r, np.float32).ravel(),
    ]).reshape(-1, 1)

    in_maps = []
    for i in range(NCORES):
        xs = np.ascontiguousarray(x[i * BS:(i + 1) * BS].T).astype(fwd_np)
        in_maps.append({
            "xt": xs, "w1t": w1t, "w2t": w2t, "w2": w2f, "bz": bz,
        })

    trace = bool(os.environ.get("KERNEL_TRACE"))
    if trace:
        _ensure_profile_hook()
    res = run_bass_kernel_spmd(nc, in_maps, core_ids=list(range(NCORES)),
                               trace=trace)

    rec = np.empty((B, D), np.float32)
    c2 = np.empty((B, H2), np.float32)
    Jst = np.empty((B, D, D), np.float32)
    for i in range(NCORES):
        r = res.results[i]
        rec[i * BS:(i + 1) * BS] = r["rec"].T.astype(np.float32)
        c2[i * BS:(i + 1) * BS] = r["c2o"].T.astype(np.float32)
        Jst[i * BS:(i + 1) * BS] = r["jac"].astype(np.float32)

    # reference's cat/reshape permutation
    Jac = np.transpose(Jst, (1, 0, 2)).reshape(D, B * D).reshape(B, D, D)
    out = (rec, c2, Jac)
    if _want_results:
        return out, res
    return out
